# revision 1
# baseline (speedup 1.0000x reference)
"""Trainium2 Bass kernel for nn_End2EndRVFixedOutput (nms_detection).

Reference semantics: out[100,7] starts at zeros; for n = 0..7 in order,
with off_n = (0 if n==0 else num_dets[n-1]) and k_n = num_dets[n],
rows [off_n, off_n+k_n) are overwritten with
[n, boxes[n,j,0:4], classes[n,j], scores[n,j]] for j = row-off_n.

num_dets < 12, so only the [:, :12] input slices matter and only out rows
0..21 can ever be written.  Device algorithm (per core, inputs replicated):

  1. x7[96,7] = [vd | boxes | classes | scores] for rows p = 12n+j is
     assembled by direct column DMAs straight from the full DRAM tensors.
  2. num_dets is cast and partition-shifted (stream_shuffle) to give per-
     batch k and off; tiny bf16 matmuls against selection constants
     broadcast them to the 96 (n,j) rows and compute, per output row r,
     batch coverage rm8[n,r] = (off_n <= r < off_n+k_n) and its suffix
     count stn[n,r] = sum_{m>n} rm8[m,r] (packed as one PSUM tile
     [stn | 4096*rm]).  Scatter targets and the last-writer gate:
        rpv[p]   = off_n + j + 1e6 + 1e6*(j >= k_n)
        a96c[p]  = stn96[p,r_p] + 4096*rm96[p,r_p]   # one-hot + accum_out
        w96[p]   = (a96c[p] == 4096)                 # covered, no later writer
        ridx[p]  = rpv[p] - 1e6*w96[p]
  3. One indirect DMA scatters x7 rows to out[ridx].  Gating makes the
     destinations UNIQUE (exactly the winning writer per row), so nothing
     relies on DMA descriptor ordering; indices >= 1e6 are skipped via
     bounds_check, leaving those rows at the runtime's zero-donated value.

All arithmetic is exact (masks are 0/1, indices are small ints), so the
output matches the reference bit-for-bit.  Every core runs the full
(tiny) computation; core 0's output is returned.  Measured on trn2:
~17.6 us HW exec per core (vs ~13.5 us for an empty DMA-through kernel
on this stack), relative error 0.0.
"""

import sys

import numpy as np

_TRN_REPO = "/opt/trn_rl_repo"
if _TRN_REPO not in sys.path:
    sys.path.insert(0, _TRN_REPO)

import ml_dtypes

import concourse.bacc as bacc
import concourse.bass as bass
import concourse.mybir as mybir
import concourse.tile as tile
from concourse.bass_utils import run_bass_kernel_spmd

B = 8          # batches
N_FULL = 8192  # detections per batch in the full input
J = 12         # num_dets < 12, so only rows [:12] of each batch matter
R = 100        # fixed output rows
P96 = B * J    # 96 stacked (batch, j) rows
OOB = 1.0e6    # pushed past bounds_check so the scatter skips the row

F32 = mybir.dt.float32
BF16 = mybir.dt.bfloat16
I32 = mybir.dt.int32

# f32 constant blob CB96 [96,3] = j96 | j96+OOB | vd96
CONST_LEN = P96 * 3
# bf16 constant blob: U96 | SEL96 | 4096*SEL96, packed per-row as [8,288]
GW = 4096.0  # weight separating the rm-half from the stn-half in the accum
CONSTBF_LEN = 8 * (3 * P96)


def _make_consts():
    p = np.arange(P96)
    m = np.arange(B)
    j96 = (p % J).astype(np.float32)[:, None]                            # [96,1]
    vd96 = (p // J).astype(np.float32)[:, None]                          # [96,1]
    blob = (
        np.concatenate([j96, j96 + OOB, vd96], axis=1).ravel().astype(np.float32)
    )
    assert blob.shape == (CONST_LEN,)
    u96 = (m[:, None] > p[None, :] // J).astype(np.float32)              # [8,96]
    sel96 = (m[:, None] == p[None, :] // J).astype(np.float32)           # [8,96]
    blobbf = (
        np.concatenate([u96, sel96, GW * sel96], axis=1)
        .ravel()
        .astype(ml_dtypes.bfloat16)
    )
    assert blobbf.shape == (CONSTBF_LEN,)
    return np.ascontiguousarray(blob), np.ascontiguousarray(blobbf)


def _build_nc() -> bass.Bass:
    nc = bacc.Bacc(None, target_bir_lowering=False, num_swdge_queues=4)
    nd_d = nc.dram_tensor("num_dets", [B], I32, kind="ExternalInput")
    boxes_d = nc.dram_tensor("boxes", [B, N_FULL, 4], F32, kind="ExternalInput")
    scores_d = nc.dram_tensor("scores", [B, N_FULL], F32, kind="ExternalInput")
    classes_d = nc.dram_tensor("classes", [B, N_FULL], F32, kind="ExternalInput")
    const_d = nc.dram_tensor("consts", [CONST_LEN], F32, kind="ExternalInput")
    constbf_d = nc.dram_tensor("constsbf", [CONSTBF_LEN], BF16, kind="ExternalInput")
    out_d = nc.dram_tensor("out", [R, 7], F32, kind="ExternalOutput")

    with tile.TileContext(nc) as tc:
        with (
            tc.tile_pool(name="sb", bufs=1) as sb,
            tc.tile_pool(name="ps", bufs=1, space=bass.MemorySpace.PSUM) as ps,
        ):
            ndi = sb.tile([B, 1], I32)
            cb96 = sb.tile([P96, 3], F32)
            r8i = sb.tile([B, R], I32)
            r2i = sb.tile([P96, 2 * R], I32)
            usel = sb.tile([B, 3 * P96], BF16)
            x7 = sb.tile([P96, 7], F32)

            k32 = sb.tile([32, 1], F32)
            off32 = sb.tile([32, 1], F32)
            k8bf = sb.tile([B, 1], BF16)
            off8bf = sb.tile([B, 1], BF16)
            s8f = sb.tile([B, 1], F32)
            u8c = sb.tile([B, R], F32)
            rm8 = sb.tile([B, R], BF16)
            b2 = sb.tile([P96, 1], F32)
            rpv = sb.tile([P96, 1], F32)
            scr200 = sb.tile([P96, 2 * R], F32)
            a96c = sb.tile([P96, 1], F32)
            w96 = sb.tile([P96, 1], F32)
            ridx = sb.tile([P96, 1], I32)

            comb = ps.tile([P96, 2 * R], F32)
            k96p = ps.tile([P96, 1], F32)
            off96p = ps.tile([P96, 1], F32)

            U96 = usel[:, 0:P96]
            SEL96 = usel[:, P96 : 2 * P96]
            SEL96W = usel[:, 2 * P96 : 3 * P96]
            J96 = cb96[:, 0:1]
            JO96 = cb96[:, 1:2]
            VD96 = cb96[:, 2:3]

            nc.gpsimd.memset(k32[:], 0.0)
            # on-device iotas replace the big row-index constants:
            # r8i[n,r] = r; r2i[p,:] = [r+OOB | r+OOB] (both accum halves)
            nc.gpsimd.iota(r8i[:], pattern=[[1, R]], base=0, channel_multiplier=0)
            nc.gpsimd.iota(
                r2i[:], pattern=[[0, 2], [1, R]], base=int(OOB), channel_multiplier=0
            )

            # loads spread over the queues; the runtime zero-donates output
            # buffers, so rows the scatter skips are already zero (no
            # explicit zero-fill needed).
            nc.sync.dma_start(out=ndi[:], in_=nd_d[:].rearrange("(p f) -> p f", f=1))
            nc.gpsimd.dma_start(out=x7[:, 5:6], in_=classes_d[:, 0:J])
            nc.scalar.dma_start(
                out=cb96[:], in_=const_d[:].rearrange("(p f) -> p f", p=P96)
            )
            nc.gpsimd.dma_start(out=x7[:, 6:7], in_=scores_d[:, 0:J])
            nc.scalar.dma_start(out=usel[:], in_=constbf_d[:].rearrange(
                "(p f) -> p f", p=B
            ))
            nc.gpsimd.dma_start(out=x7[:, 1:5], in_=boxes_d[:, 0:J, :])

            alu = mybir.AluOpType
            vec = nc.vector

            # critical chain first: k32[0:8] = float(num_dets);
            # off32[n] = k32[n-1] via partition shift; coverage masks
            vec.tensor_copy(k32[0:B, :], ndi[:])
            vec.stream_shuffle(off32[:], k32[:], mask=[31] + list(range(31)))
            vec.tensor_tensor(s8f[:], k32[0:B, :], off32[0:B, :], alu.add)
            vec.tensor_scalar(u8c[:], r8i[:], off32[0:B, :], None, alu.is_ge)
            vec.scalar_tensor_tensor(
                rm8[:], r8i[:], s8f[:], u8c[:], alu.is_lt, alu.mult
            )
            # vd column of x7 (scalar engine: DVE is the busy one)
            nc.scalar.copy(x7[:, 0:1], VD96)
            # bf16 casts + broadcasts of k/off to the 96 (n,j) rows; these
            # feed b2/rpv which are only needed after the comb matmuls
            vec.tensor_copy(k8bf[:], k32[0:B, :])
            vec.tensor_copy(off8bf[:], off32[0:B, :])
            nc.tensor.matmul(k96p[:], SEL96, k8bf[:], start=True, stop=True)
            nc.tensor.matmul(off96p[:], SEL96, off8bf[:], start=True, stop=True)
            # two parallel matmuls into one PSUM tile: cols 0:100 hold
            # stn96[p,r] = sum_{m>n} rm8[m,r], cols 100:200 hold GW*rm8[n,r]
            nc.tensor.matmul(comb[:, 0:R], U96, rm8[:], start=True, stop=True)
            nc.tensor.matmul(comb[:, R : 2 * R], SEL96W, rm8[:], start=True, stop=True)

            # per-(n,j) scatter targets (fills DVE gaps while PE runs);
            # rpv = off + j + OOB + OOB*(j >= k)
            vec.tensor_scalar(b2[:], k96p[:], J96, OOB, alu.is_le, alu.mult)
            vec.scalar_tensor_tensor(
                rpv[:], off96p[:], JO96, b2[:], alu.add, alu.add
            )

            # one-hot extraction of both halves at r+OOB = rpv[p]:
            # a96c[p] = stn96[p,r_p] + GW*rm96[p,r_p]; winner iff == GW
            vec.scalar_tensor_tensor(
                scr200[:], r2i[:], rpv[:], comb[:], alu.is_equal, alu.mult,
                accum_out=a96c[:],
            )
            vec.tensor_scalar(w96[:], a96c[:], GW, None, alu.is_equal)
            # ridx = rpv - OOB*w96: winners land on their row, rest stay OOB
            vec.scalar_tensor_tensor(
                ridx[:], w96[:], -OOB, rpv[:], alu.mult, alu.add
            )

            # winner-only scatter: destinations are unique, no ordering needed
            nc.gpsimd.indirect_dma_start(
                out=out_d[:],
                out_offset=bass.IndirectOffsetOnAxis(ap=ridx[:], axis=0),
                in_=x7[:],
                in_offset=None,
                bounds_check=R - 1,
                oob_is_err=False,
            )

    nc.finalize()
    return nc


_CACHE: dict = {}


def _get_built():
    if "nc" not in _CACHE:
        _CACHE["nc"] = _build_nc()
        _CACHE["consts"] = _make_consts()
    return _CACHE["nc"], _CACHE["consts"]


def run(inputs: dict, trace: bool = False, **spmd_kwargs):
    """Run on all 8 cores with replicated inputs; returns (out, BassKernelResults)."""
    nc, (consts, constsbf) = _get_built()
    in_map = {
        "num_dets": np.ascontiguousarray(inputs["num_dets"], dtype=np.int32),
        "boxes": np.ascontiguousarray(inputs["boxes"], dtype=np.float32),
        "scores": np.ascontiguousarray(inputs["scores"], dtype=np.float32),
        "classes": np.ascontiguousarray(inputs["classes"], dtype=np.float32),
        "consts": consts,
        "constsbf": constsbf,
    }
    res = run_bass_kernel_spmd(
        nc,
        [dict(in_map) for _ in range(8)],
        core_ids=list(range(8)),
        trace=trace,
        **spmd_kwargs,
    )
    return res.results[0]["out"], res


def kernel(num_dets, boxes, scores, classes):
    out, _ = run(
        {"num_dets": num_dets, "boxes": boxes, "scores": scores, "classes": classes}
    )
    return out



# revision 2
# speedup vs baseline: 1.0366x; 1.0366x over previous
"""Trainium2 Bass kernel for nn_End2EndRVFixedOutput (nms_detection).

Reference semantics: out[100,7] starts at zeros; for n = 0..7 in order,
with off_n = (0 if n==0 else num_dets[n-1]) and k_n = num_dets[n],
rows [off_n, off_n+k_n) are overwritten with
[n, boxes[n,j,0:4], classes[n,j], scores[n,j]] for j = row-off_n.

num_dets < 12, so only the [:, :12] input slices matter and only out rows
0..21 can ever be written.  Device algorithm (per core, inputs replicated):

  x7[96,7] rows p=(n,j) are assembled by column DMAs from the full DRAM
  tensors.  The dependent chain from num_dets is kept minimal:

    k = f32(num_dets); off = shift(k)                 (DVE)
    rm16[n,r] = 16*(off_n <= r < off_n+k_n)  [8,24]   (DVE, bf16)
    off96p = SEL96 @ off_bf16                 [96,1]   (PE)
    comb   = [U16 | SEL96] @ rm16             [96,48]  (PE; stn | 16*rm)
    rpv16  = off96p + (j + 16e6)              [96,1]   (DVE)
    a96    = sum_c (r2i16==rpv16)*comb        [96,1]   (DVE one-hot accum)
    ridx   = |(-1e6)*a96 + rpv16|             [96,1]   (ACT Abs, i32 out)

  a96 = stn + 16*rm at the row this p targets; the winner (last writer:
  rm=1, stn=0) has a96 == 16, so Abs collapses to the target row r_p;
  every loser lands >= 999978 and is skipped by the scatter's
  bounds_check.  One indirect DMA scatters x7 rows to out[ridx]
  (destinations unique).  All arithmetic is exact; output matches the
  reference bit-for-bit.  Rows never written stay at the runtime's
  zero-donated value.  Every core runs the full computation; core 0's
  output is returned.
"""

import sys

import numpy as np

_TRN_REPO = "/opt/trn_rl_repo"
if _TRN_REPO not in sys.path:
    sys.path.insert(0, _TRN_REPO)

import ml_dtypes

import concourse.bacc as bacc
import concourse.bass as bass
import concourse.mybir as mybir
import concourse.tile as tile
from concourse.bass_utils import run_bass_kernel_spmd

B = 8          # batches
N_FULL = 8192  # detections per batch in the full input
J = 12         # num_dets < 12, so only rows [:12] of each batch matter
R = 100        # fixed output rows
R24 = 24       # rows 0..21 are the only reachable targets
P96 = B * J    # 96 stacked (batch, j) rows
OOB = 1.0e6    # loser offset; pushed past bounds_check so scatter skips
BIG = 16.0 * OOB  # 16e6 < 2^24, keeps all index arithmetic exact in f32

F32 = mybir.dt.float32
BF16 = mybir.dt.bfloat16
I32 = mybir.dt.int32

CF_COLS = 50      # [96,50] f32 blob: r2i16 (48) | j16o (1) | vd (1)
UBR_COLS = 216    # [8,216] bf16 blob: U16 (96) | SEL96 (96) | r8 ramp (24)


def _make_consts():
    p = np.arange(P96)
    m = np.arange(B)
    c = np.arange(2 * R24)
    r2i16 = BIG + (c % R24).astype(np.float32)                       # [48]
    cf = np.empty((P96, CF_COLS), dtype=np.float32)
    cf[:, 0 : 2 * R24] = r2i16[None, :]
    cf[:, 2 * R24] = (p % J).astype(np.float32) + BIG                # j16o
    cf[:, 2 * R24 + 1] = (p // J).astype(np.float32)                 # vd
    u16 = (m[:, None] > p[None, :] // J).astype(np.float32) / 16.0   # [8,96]
    sel = (m[:, None] == p[None, :] // J).astype(np.float32)         # [8,96]
    r8 = np.broadcast_to(np.arange(R24, dtype=np.float32), (B, R24))
    ubr = np.concatenate([u16, sel, r8], axis=1).astype(ml_dtypes.bfloat16)
    assert ubr.shape == (B, UBR_COLS)
    return (
        np.ascontiguousarray(cf.ravel()),
        np.ascontiguousarray(ubr.ravel()),
    )


def _build_nc() -> bass.Bass:
    nc = bacc.Bacc(None, target_bir_lowering=False, num_swdge_queues=4)
    nd_d = nc.dram_tensor("num_dets", [B], I32, kind="ExternalInput")
    boxes_d = nc.dram_tensor("boxes", [B, N_FULL, 4], F32, kind="ExternalInput")
    scores_d = nc.dram_tensor("scores", [B, N_FULL], F32, kind="ExternalInput")
    classes_d = nc.dram_tensor("classes", [B, N_FULL], F32, kind="ExternalInput")
    cf_d = nc.dram_tensor("constf", [P96 * CF_COLS], F32, kind="ExternalInput")
    ubr_d = nc.dram_tensor("constbf", [B * UBR_COLS], BF16, kind="ExternalInput")
    out_d = nc.dram_tensor("out", [R, 7], F32, kind="ExternalOutput")

    with tile.TileContext(nc) as tc:
        with (
            tc.tile_pool(name="sb", bufs=1) as sb,
            tc.tile_pool(name="ps", bufs=1, space=bass.MemorySpace.PSUM) as ps,
        ):
            ndi = sb.tile([B, 1], I32)
            koff = sb.tile([32, 2], F32)          # col0 = k, col1 = off
            s8f = sb.tile([B, 1], F32)
            u8c = sb.tile([B, R24], F32)
            rm16 = sb.tile([B, R24], BF16)
            off8bf = sb.tile([B, 1], BF16)
            cf = sb.tile([P96, CF_COLS], F32)
            ubr = sb.tile([B, UBR_COLS], BF16)
            x7 = sb.tile([P96, 7], F32)
            rpv16 = sb.tile([P96, 1], F32)
            scr = sb.tile([P96, 2 * R24], F32)
            a96 = sb.tile([P96, 1], F32)
            ridx = sb.tile([P96, 1], I32)

            comb = ps.tile([P96, 2 * R24], F32)
            off96p = ps.tile([P96, 1], F32)

            U16 = ubr[:, 0:P96]
            SEL96 = ubr[:, P96 : 2 * P96]
            R8 = ubr[:, 2 * P96 : 2 * P96 + R24]
            R2I16 = cf[:, 0 : 2 * R24]
            J16O = cf[:, 2 * R24 : 2 * R24 + 1]
            VD96 = cf[:, 2 * R24 + 1 : 2 * R24 + 2]

            alu = mybir.AluOpType
            vec = nc.vector

            # --- input DMAs.  num_dets first on SP (lowest latency); the
            # const blobs ride the other first-slot queues so every piece
            # of the dependent chain is resident before num_dets lands.
            nc.sync.dma_start(out=ndi[:], in_=nd_d[:].rearrange("(p f) -> p f", f=1))
            nc.gpsimd.dma_start(
                out=ubr[:], in_=ubr_d[:].rearrange("(p f) -> p f", p=B)
            )
            nc.scalar.dma_start(
                out=cf[:], in_=cf_d[:].rearrange("(p f) -> p f", p=P96)
            )
            nc.sync.dma_start(out=x7[:, 1:5], in_=boxes_d[:, 0:J, :])
            nc.gpsimd.dma_start(out=x7[:, 5:6], in_=classes_d[:, 0:J])
            nc.gpsimd.dma_start(out=x7[:, 6:7], in_=scores_d[:, 0:J])

            # --- dependent chain (DVE unless noted)
            vec.memset(koff[:], 0.0)  # partitions 8..31 must read as 0
            vec.tensor_copy(koff[0:B, 0:1], ndi[:])          # k = f32(nd)
            vec.stream_shuffle(
                koff[:, 1:2], koff[:, 0:1], mask=[31] + list(range(31))
            )                                                 # off_n = k_{n-1}
            vec.tensor_tensor(
                s8f[:], koff[0:B, 0:1], koff[0:B, 1:2], alu.add
            )                                                 # off + k
            # u8c = 16*(r >= off); rm16 = (r < off+k) * u8c  (bf16, {0,16})
            vec.tensor_scalar(
                u8c[:], R8, koff[0:B, 1:2], 16.0, alu.is_ge, alu.mult
            )
            vec.scalar_tensor_tensor(
                rm16[:], R8, s8f[:], u8c[:], alu.is_lt, alu.mult
            )
            # off broadcast to 96 rows via PE; cast on ACT to keep DVE free
            nc.scalar.copy(off8bf[:], koff[0:B, 1:2])
            nc.tensor.matmul(off96p[:], SEL96, off8bf[:], start=True, stop=True)
            # comb cols 0:24 = stn (U16 un-scales the 16), cols 24:48 = 16*rm
            nc.tensor.matmul(
                comb[:, R24 : 2 * R24], SEL96, rm16[:], start=True, stop=True
            )
            nc.tensor.matmul(comb[:, 0:R24], U16, rm16[:], start=True, stop=True)
            # x7 vd column (ACT, off critical path)
            nc.scalar.copy(x7[:, 0:1], VD96)

            # rpv16 = off + j + 16e6; one-hot extract a96 = stn + 16*rm
            vec.tensor_tensor(rpv16[:], off96p[:], J16O, alu.add)
            vec.scalar_tensor_tensor(
                scr[:], R2I16, rpv16[:], comb[:], alu.is_equal, alu.mult,
                accum_out=a96[:],
            )
            # winner (a96==16) -> r_p; losers -> >= 999978 (skipped)
            nc.scalar.activation(
                ridx[:], a96[:], mybir.ActivationFunctionType.Abs,
                bias=rpv16[:, 0:1], scale=-OOB,
            )

            # winner-only scatter: destinations unique, no ordering needed
            nc.gpsimd.indirect_dma_start(
                out=out_d[:],
                out_offset=bass.IndirectOffsetOnAxis(ap=ridx[:], axis=0),
                in_=x7[:],
                in_offset=None,
                bounds_check=R - 1,
                oob_is_err=False,
            )

    nc.finalize()
    return nc


_CACHE: dict = {}


def _get_built():
    if "nc" not in _CACHE:
        _CACHE["nc"] = _build_nc()
        _CACHE["consts"] = _make_consts()
    return _CACHE["nc"], _CACHE["consts"]


def run(inputs: dict, trace: bool = False, **spmd_kwargs):
    """Run on all 8 cores with replicated inputs; returns (out, BassKernelResults)."""
    nc, (cf, ubr) = _get_built()
    in_map = {
        "num_dets": np.ascontiguousarray(inputs["num_dets"], dtype=np.int32),
        "boxes": np.ascontiguousarray(inputs["boxes"], dtype=np.float32),
        "scores": np.ascontiguousarray(inputs["scores"], dtype=np.float32),
        "classes": np.ascontiguousarray(inputs["classes"], dtype=np.float32),
        "constf": cf,
        "constbf": ubr,
    }
    res = run_bass_kernel_spmd(
        nc,
        [dict(in_map) for _ in range(8)],
        core_ids=list(range(8)),
        trace=trace,
        **spmd_kwargs,
    )
    return res.results[0]["out"], res


def kernel(num_dets, boxes, scores, classes):
    out, _ = run(
        {"num_dets": num_dets, "boxes": boxes, "scores": scores, "classes": classes}
    )
    return out
